# revision 1
# baseline (speedup 1.0000x reference)
"""Multi-head attention (B=4, N=2048, C=768, H=12) on 8 Trainium2 NeuronCores.

Sharding: core c = (batch b = c//2, head-group g = c%2 of 6 heads).
Each core: qkv projection for its (b, g), attention for 6 heads, partial
output projection against w_proj[:, g-cols]. Host sums the two partial
projections per batch, adds bias, transposes. No collectives.

Device layouts (everything pre-transposed on host; no on-device transposes):
  xT   [768, 2048]   x[b].T              (c on partitions)
  wqk  [768, 768]    [w_q_g; w_k_g].T    (c on partitions, o free)
  wv   [768, 384]    w_v_g.T
  wp   [384, 768]    w_proj[:, gcols].T  (f on partitions, o free)
  out  [768, 2048]   partial (w_proj_g @ attn_out).T  fp32

Attention per head h (d=64), transposed-score ("sT") formulation:
  sT[k, q] = (k_h chunk).T-matmul: lhsT = k_h [64, 128], rhs = q_h [64, 1024]
  eT = exp(sT * 1/8)  (ScalarE, psum -> sbuf, no max-subtraction: scores ~ N(0,1))
  PV: lhsT = vT chunk [128, 65] (col 64 = ones -> softmax denominator in row 64),
      rhs = eT [128, 1024], accumulate over 16 chunks -> C [65, 1024] psum
  out_h = C[0:64] * reciprocal(C[64]) broadcast over partitions (DMA bcast)
"""

import sys

for _p in ("/opt/trn_rl_repo", "/root/.axon_site/_ro/trn_rl_repo"):
    if _p not in sys.path:
        sys.path.insert(0, _p)

import numpy as np
import ml_dtypes

import concourse.bass as bass
import concourse.bacc as bacc
import concourse.mybir as mybir
import concourse.tile as tile
from concourse.bass_utils import run_bass_kernel_spmd

B, N, C = 4, 2048, 768
H, D = 12, 64
HG = 6          # heads per core
P = 128
NCORES = 8
CK = C // P     # 6 contraction chunks for qkv
NT = N // P     # 16 token chunks
QG = 2          # q-groups of 1024
QW = N // QG    # 1024
SCALE = D ** -0.5

# precision plan: qkv inputs and q/k (the exp-amplified path) in float32r
# (TF32-like, ~7e-5 matmul relerr); the averaging-dominated PV/proj in bf16
R_DT = mybir.dt.float32r
B_DT = mybir.dt.bfloat16
NP_BF = ml_dtypes.bfloat16

# DVE bit-trick exp(s/8) = 2^(s*c): z = fl(y+M) rounds y; v = -frac(y);
# 2^frac ~ c2*(v+b)^2 + d (quadratic, ~2.2e-3 rel, under eT's bf16 rounding);
# 2^int via (z_bits<<23) + (127<<23). HW-validated in smoke3.py.
DVX_C = float(np.log2(np.e) / 8.0)
DVX_M = 12582912.0            # 1.5 * 2^23 round-to-int magic
DVX_C2 = 0.239449394
DVX_B = -1.473168703
DVX_D = 0.480870388
DVX_EBIAS = 127 << 23         # exactly representable in f32 immediates

_CACHED_NC = None


def build_nc():
    nc = bacc.Bacc("TRN2", target_bir_lowering=False, debug=False, num_devices=NCORES)
    f32 = mybir.dt.float32

    # inputs arrive pre-tiled on host: partition-major [P, chunks, free]
    xT = nc.declare_dram_parameter("xT", [P, CK, N], R_DT, isOutput=False)
    wqk = nc.declare_dram_parameter("wqk", [P, CK, 2 * HG * D], R_DT, isOutput=False)
    wv = nc.declare_dram_parameter("wv", [P, CK, HG * D], R_DT, isOutput=False)
    wp = nc.declare_dram_parameter("wp", [P, HG * D // P, C], B_DT, isOutput=False)
    out = nc.declare_dram_parameter("out", [C, N], f32, isOutput=True)

    with tile.TileContext(nc) as tc:
        with (
            tc.tile_pool(name="big", bufs=1) as big,
            tc.tile_pool(name="et", bufs=19) as etp,
            tc.tile_pool(name="nrm", bufs=2) as nrm,
            tc.tile_pool(name="stg", bufs=4) as stg,
            tc.tile_pool(name="psa", bufs=2, space="PSUM") as psa,
            tc.tile_pool(name="psc", bufs=2, space="PSUM") as psc,
        ):
            # ---------------- loads ----------------
            # split per contraction chunk so qkv matmuls can start early
            xT_sb = big.tile([P, CK, N], R_DT)
            wqk_sb = big.tile([P, CK, 2 * HG * D], R_DT)
            wv_sb = big.tile([P, CK, HG * D], R_DT)
            for kc in range(CK):
                nc.sync.dma_start(wqk_sb[:, kc], wqk[:, kc])
                nc.sync.dma_start(xT_sb[:, kc], xT[:, kc])
            nc.sync.dma_start(wv_sb, wv[:, :, :])
            wp_sb = big.tile([P, HG * D // P, C], B_DT)
            nc.sync.dma_start(wp_sb, wp[:, :, :])

            # warm the ACT exp table (and engine) during the load phase so the
            # first real exp doesn't pay the ~2.7us table-load latency
            warm = nrm.tile([1, 32], f32, tag="warm")
            nc.vector.memset(warm, 0.0)
            nc.scalar.activation(warm, warm, mybir.ActivationFunctionType.Exp,
                                 bias=0.0, scale=1.0)

            # qk[o, n]: o = 6 q-head cols then 6 k-head cols -> 6 partition blocks
            qk_sb = big.tile([P, 2 * HG * D // P, N], R_DT)
            # vT[n, f] with per-head ones column: [n, 6*65], col h*65+64 == 1.0
            vT_sb = big.tile([P, NT, HG * (D + 1)], B_DT)
            ones_view = vT_sb.rearrange("p n (h s) -> p n h s", s=D + 1)[:, :, :, D : D + 1]
            nc.vector.memset(ones_view, 1.0)
            # attention outputs [f, n], f = (head, d) -> 3 partition blocks
            out_h = big.tile([P, HG * D // P, N], B_DT)

            # ---------------- qkv ----------------
            def emit_qk_group(ot, nh, pool, tagname):
                ps = pool.tile([P, QW], f32, tag=tagname, name=f"qk_ps{ot}_{nh}")
                for kc in range(CK):
                    for i in range(QW // 512):
                        nc.tensor.matmul(
                            ps[:, i * 512 : (i + 1) * 512],
                            lhsT=wqk_sb[:, kc, ot * P : (ot + 1) * P],
                            rhs=xT_sb[:, kc, nh * QW + i * 512 : nh * QW + (i + 1) * 512],
                            start=(kc == 0),
                            stop=(kc == CK - 1),
                        )
                nc.vector.tensor_copy(qk_sb[:, ot, nh * QW : (nh + 1) * QW], ps)

            def emit_vt_group(nt):
                ps = psc.tile([P, HG * D], f32, tag="c", name=f"vt_ps{nt}")
                for kc in range(CK):
                    nc.tensor.matmul(
                        ps,
                        lhsT=xT_sb[:, kc, nt * P : (nt + 1) * P],
                        rhs=wv_sb[:, kc, :],
                        start=(kc == 0),
                        stop=(kc == CK - 1),
                    )
                nc.vector.tensor_copy(
                    vT_sb.rearrange("p n (h s) -> p n h s", s=D + 1)[:, nt, :, 0:D],
                    ps.rearrange("p (h s) -> p h s", s=D),
                )

            # pair 0's q and k blocks upfront, kc-OUTER across all four
            # accumulators (both psum pools are free at startup) so each
            # group's matmul fires as soon as its xT chunk lands -- only the
            # final kc's matmuls remain after the last input DMA
            up_blks = [(0, 0), (HG * D // P, 0), (0, 1), (HG * D // P, 1)]
            up_pools = [psa, psa, psc, psc]
            up_tags = ["a", "a", "c", "c"]
            up_ps = [
                pool.tile([P, QW], f32, tag=tag, name=f"up_ps{j}")
                for j, (pool, tag) in enumerate(zip(up_pools, up_tags))
            ]
            for kc in range(CK):
                for j, (ot, nh) in enumerate(up_blks):
                    for i in range(QW // 512):
                        nc.tensor.matmul(
                            up_ps[j][:, i * 512 : (i + 1) * 512],
                            lhsT=wqk_sb[:, kc, ot * P : (ot + 1) * P],
                            rhs=xT_sb[:, kc, nh * QW + i * 512 : nh * QW + (i + 1) * 512],
                            start=(kc == 0),
                            stop=(kc == CK - 1),
                        )
            for j, (ot, nh) in enumerate(up_blks):
                # the nh0 copies (j=0,1) gate the first scores: run one on the
                # idle ScalarE and one on DVE so they complete in parallel
                if j in (0, 3):
                    nc.scalar.copy(qk_sb[:, ot, nh * QW : (nh + 1) * QW], up_ps[j])
                else:
                    nc.vector.tensor_copy(qk_sb[:, ot, nh * QW : (nh + 1) * QW], up_ps[j])

            # deferred work per (pair, qg) window: callables emitted just
            # before the PV accumulator is allocated
            # each deferred-qkv burst is split by n-half: the nh0 halves are
            # needed just before pair p's first scores, the nh1 halves only 8
            # chunks later -- two small bursts stay under the exp backlog
            windows = {
                (0, 0): [(lambda nt=nt: emit_vt_group(nt)) for nt in range(NT)],
                (0, 1): [
                    (lambda blk=blk: emit_qk_group(blk, 0, psc, "c"))
                    for blk in (1, HG * D // P + 1)
                ],
                (1, 0): [
                    (lambda blk=blk: emit_qk_group(blk, 1, psc, "c"))
                    for blk in (1, HG * D // P + 1)
                ],
                (1, 1): [
                    (lambda blk=blk: emit_qk_group(blk, 0, psc, "c"))
                    for blk in (2, HG * D // P + 2)
                ],
                (2, 0): [
                    (lambda blk=blk: emit_qk_group(blk, 1, psc, "c"))
                    for blk in (2, HG * D // P + 2)
                ],
            }
            def emit_proj_group(ot, nh, pool, tagname):
                ps = pool.tile([P, QW], f32, tag=tagname, name=f"pj_ps{ot}_{nh}")
                for fc in range(HG * D // P):
                    for i in range(QW // 512):
                        nc.tensor.matmul(
                            ps[:, i * 512 : (i + 1) * 512],
                            lhsT=wp_sb[:, fc, ot * P : (ot + 1) * P],
                            rhs=out_h[:, fc, nh * QW + i * 512 : nh * QW + (i + 1) * 512],
                            start=(fc == 0),
                            stop=(fc == HG * D // P - 1),
                        )
                for i in range(QW // 512):
                    so = stg.tile([P, 512], f32, tag="so", name=f"so{ot}_{nh}_{i}")
                    nc.vector.tensor_copy(so, ps[:, i * 512 : (i + 1) * 512])
                    nc.sync.dma_start(
                        out[ot * P : (ot + 1) * P, nh * QW + i * 512 : nh * QW + (i + 1) * 512], so
                    )

            # proj for the first q-half runs inside pair 2's last q-group,
            # where out_h[:, :, 0:QW] is already complete
            windows[(2, 1)] = [
                (lambda ot=ot: emit_proj_group(ot, 0, psc, "c")) for ot in range(C // P)
            ]

            # ---------------- attention ----------------
            # head pairs (2p, 2p+1): even head lives at partitions 0-63 of its
            # qk block, odd head at 64-127.  Their K=64 score matmuls go to
            # disjoint PE row-groups (tile_position auto from base_partition)
            # and run concurrently when issued back-to-back.
            #
            # The scores/exp stream is one global software pipeline that runs
            # PRE chunks ahead of PV across ALL (pair, qg) segments, so the
            # ScalarE exp backlog never drains at segment boundaries and
            # covers the deferred qkv/vT/proj bursts in the windows.
            PRE = 8
            segs = [(p_, qg) for p_ in range(HG // 2) for qg in range(QG)]
            score_queue = [(p_, qg, ch) for (p_, qg) in segs for ch in range(NT)]
            et_tiles = {}
            qpos = [0]

            def emit_scores(sp, sqg, ch):
                qb = sp
                kb = HG * D // P + sp
                aps2 = [psa.tile([P, QW], f32, tag="a", name=f"aps{sp}_{sqg}_{ch}_{e}") for e in range(2)]
                # alternate even/odd so consecutive matmuls hit
                # disjoint PE row-groups and run concurrently
                for i in range(QW // 512):
                    for e in range(2):
                        base = e * D
                        nc.tensor.matmul(
                            aps2[e][:, i * 512 : (i + 1) * 512],
                            lhsT=qk_sb[base : base + D, kb, ch * P : (ch + 1) * P],
                            rhs=qk_sb[base : base + D, qb, sqg * QW + i * 512 : sqg * QW + (i + 1) * 512],
                            start=True,
                            stop=True,
                        )
                for e in range(2):
                    eT = etp.tile([P, QW], B_DT, tag="et", name=f"et{sp}_{sqg}_{ch}_{e}")
                    nc.scalar.activation(
                        eT, aps2[e], mybir.ActivationFunctionType.Exp,
                        bias=0.0, scale=float(SCALE),
                    )
                    et_tiles[(sp, sqg, ch, e)] = eT

            def pump_scores(n):
                for _ in range(n):
                    if qpos[0] < len(score_queue):
                        emit_scores(*score_queue[qpos[0]])
                        qpos[0] += 1

            pump_scores(PRE)
            for p_, qg in segs:
                for work in windows.get((p_, qg), []):
                    work()
                cps2 = [psc.tile([P, QW], f32, tag="c", name=f"cps{p_}_{qg}_{e}") for e in range(2)]
                for ch in range(NT):
                    pump_scores(1)
                    for e in range(2):
                        h = 2 * p_ + e
                        eT = et_tiles.pop((p_, qg, ch, e))
                        for i in range(QW // 512):
                            nc.tensor.matmul(
                                cps2[e][0 : D + 1, i * 512 : (i + 1) * 512],
                                lhsT=vT_sb[:, ch, h * (D + 1) : (h + 1) * (D + 1)],
                                rhs=eT[:, i * 512 : (i + 1) * 512],
                                start=(ch == 0),
                                stop=(ch == NT - 1),
                            )
                for e in range(2):
                    h = 2 * p_ + e
                    base = e * D
                    # normalize: out_h = C[0:64] * (1 / C[64]) bcast over partitions
                    recb = nrm.tile([D, QW], f32, tag="recb")
                    nc.vector.reciprocal(recb[0:1, :], cps2[e][D : D + 1, :])
                    nc.gpsimd.partition_broadcast(recb, recb[0:1, :])
                    nc.vector.tensor_tensor(
                        out_h[base : base + D, p_, qg * QW : (qg + 1) * QW],
                        cps2[e][0:D, :],
                        recb,
                        mybir.AluOpType.mult,
                    )


            # ---------------- proj (nh=1 half; nh=0 ran in the (2,1) window) ----
            # two-stage over ot pairs: both groups' fc0/fc1 matmuls are emitted
            # before either normalize-gated fc2, so the PE isn't stalled
            # in-order behind the last pair's normalize
            def proj_head(ot):
                ps = psa.tile([P, QW], f32, tag="a", name=f"pj_ps{ot}_1")
                for fc in range(2):
                    for i in range(QW // 512):
                        nc.tensor.matmul(
                            ps[:, i * 512 : (i + 1) * 512],
                            lhsT=wp_sb[:, fc, ot * P : (ot + 1) * P],
                            rhs=out_h[:, fc, QW + i * 512 : QW + (i + 1) * 512],
                            start=(fc == 0),
                            stop=False,
                        )
                return ps

            def proj_tail(ot, ps):
                fc = 2
                for i in range(QW // 512):
                    nc.tensor.matmul(
                        ps[:, i * 512 : (i + 1) * 512],
                        lhsT=wp_sb[:, fc, ot * P : (ot + 1) * P],
                        rhs=out_h[:, fc, QW + i * 512 : QW + (i + 1) * 512],
                        start=False,
                        stop=True,
                    )
                for i in range(QW // 512):
                    so = stg.tile([P, 512], f32, tag="so", name=f"so{ot}_1_{i}")
                    nc.vector.tensor_copy(so, ps[:, i * 512 : (i + 1) * 512])
                    nc.sync.dma_start(
                        out[ot * P : (ot + 1) * P, QW + i * 512 : QW + (i + 1) * 512], so
                    )

            for ot in range(0, C // P, 2):
                ps0 = proj_head(ot)
                ps1 = proj_head(ot + 1)
                proj_tail(ot, ps0)
                proj_tail(ot + 1, ps1)
    nc.compile()
    return nc


def _get_nc():
    global _CACHED_NC
    if _CACHED_NC is None:
        _CACHED_NC = build_nc()
    return _CACHED_NC


def shard_inputs(x, w_qkv, w_proj):
    """Build per-core input maps from full inputs."""
    in_maps = []
    for c in range(NCORES):
        b, g = divmod(c, 2)
        r = slice(HG * D * g, HG * D * (g + 1))
        def ptile(m):
            return np.ascontiguousarray(m.reshape(m.shape[0] // P, P, m.shape[1]).transpose(1, 0, 2))
        xT = ptile(x[b].T)
        wq = w_qkv[r]
        wk = w_qkv[C + HG * D * g : C + HG * D * (g + 1)]
        wv_ = w_qkv[2 * C + HG * D * g : 2 * C + HG * D * (g + 1)]
        wqk = ptile(np.concatenate([wq, wk], axis=0).T)
        wvT = ptile(wv_.T)
        wpT = ptile(w_proj[:, r].T.astype(NP_BF))
        in_maps.append({"xT": xT, "wqk": wqk, "wv": wvT, "wp": wpT})
    return in_maps


def run(x, w_qkv, w_proj, b_proj, trace=False):
    nc = _get_nc()
    in_maps = shard_inputs(x, w_qkv, w_proj)
    try:
        res = run_bass_kernel_spmd(nc, in_maps, list(range(NCORES)), trace=trace)
    except Exception:
        # one retry for transient runtime/tunnel hiccups
        res = run_bass_kernel_spmd(nc, in_maps, list(range(NCORES)), trace=trace)
    y = np.empty((B, N, C), np.float32)
    for b in range(B):
        part = res.results[2 * b]["out"] + res.results[2 * b + 1]["out"]
        y[b] = part.T + b_proj.astype(np.float32)
    return y, res


def kernel(x, w_qkv, w_proj, b_proj):
    x = np.asarray(x, dtype=np.float32)
    w_qkv = np.asarray(w_qkv, dtype=np.float32)
    w_proj = np.asarray(w_proj, dtype=np.float32)
    b_proj = np.asarray(b_proj, dtype=np.float32)
    y, _ = run(x, w_qkv, w_proj, b_proj, trace=False)
    return y

